# revision 1
# baseline (speedup 1.0000x reference)
"""MoE top-1 routing kernel for Trainium2 (8 NeuronCores, data-parallel).

Computes, for each token t:
    clean   = input[t] @ w_gate                    # [3]
    raw     = input[t] @ w_noise                   # [3]
    logits  = clean + noise[t] * (softplus(raw) + 0.2)
    out[t]  = argmax(logits)                       # int32, first-max tie-break

Sharding: token dim split evenly across 8 cores; [256,3] weights replicated.

Per-core dataflow (32768 tokens):
  - tokens processed in 4 super-groups of 8192; token n = g*8192 + p*64 + s
    (p = SBUF partition, s = sub-tile index) so noise / output DMAs are
    contiguous per partition.
  - input tiles [128 tok, 256 d] are PE-transposed (via identity matmul) to
    [256 d, 128 tok], copied PSUM->SBUF, then matmul'd against the
    concatenated [256, 6] (w_gate | w_noise) weights, accumulating over the
    two 128-row K chunks into PSUM [128 tok, 6] slices.
  - epilogue per super-group: softplus on ACT, noise scale + add + 3-way
    argmax as arithmetic on DVE, cast to int32, DMA out.
"""

import numpy as np

N = 262144
D = 256
E = 3
NCORES = 8
NPC = N // NCORES          # 32768 tokens per core
SG = 4                     # super-groups per core
ST = 64                    # 128-token sub-tiles per super-group
BLK = 8                    # input-DMA blocks per super-group
JB = 8                     # sub-tiles per input-DMA block (1 MiB per DMA)
NOISE_EPS = 0.2

_CACHE = {}


def _build(variant="full", repeat=1, softplus_mode="stable", copy_split="any",
           small_engine="vector", in_bufs=3, tp_bufs=4, inT_bufs=6, op_bufs=2,
           jb=None, dma_engines=("sync",), tp_batch=1, fuse_io=False,
           tp_f32r=False):
    from contextlib import ExitStack

    import concourse.bacc as bacc
    import concourse.mybir as mybir
    import concourse.tile as tile
    from concourse import masks

    dt = mybir.dt
    Alu = mybir.AluOpType
    Act = mybir.ActivationFunctionType
    do_transpose = variant in ("full", "no_epilogue", "no_matmul")
    do_matmul = variant in ("full", "no_epilogue")
    do_epilogue = variant == "full"

    nc = bacc.Bacc(
        "TRN2",
        target_bir_lowering=False,
        debug=False,
        enable_asserts=False,
        num_devices=NCORES,
    )
    inp = nc.dram_tensor("input", [NPC, D], dt.float32, kind="ExternalInput").ap()
    wg = nc.dram_tensor("w_gate", [D, E], dt.float32, kind="ExternalInput").ap()
    wn = nc.dram_tensor("w_noise", [D, E], dt.float32, kind="ExternalInput").ap()
    noi = nc.dram_tensor("noise", [NPC, E], dt.float32, kind="ExternalInput").ap()
    out = nc.dram_tensor("out", [NPC], dt.int32, kind="ExternalOutput").ap()

    jb = JB if jb is None else jb
    blk = ST // jb
    # token n = g*8192 + p*64 + b*jb + j  (p: partition, st = b*jb+j: sub-tile)
    inp_r = inp.rearrange("(g p b j) d -> g b p (j d)", g=SG, p=128, b=blk, j=jb)
    noi_r = noi.rearrange("(g p s) e -> g p (s e)", g=SG, p=128, s=ST)
    out_r = out.rearrange("(g p s) -> g p s", g=SG, p=128, s=ST)
    # fused-I/O layouts: partition p, free (g, s[, e])
    noi_f = noi.rearrange("(g p s) e -> p g (s e)", g=SG, p=128, s=ST)
    out_f = out.rearrange("(g p s) -> p g s", g=SG, p=128, s=ST)

    with tile.TileContext(nc) as tc, ExitStack() as ctx:
        const_pool = ctx.enter_context(tc.tile_pool(name="const", bufs=1))
        in_pool = ctx.enter_context(tc.tile_pool(name="inp", bufs=in_bufs))
        tpsum_pool = ctx.enter_context(tc.tile_pool(name="tpsum", bufs=tp_bufs, space="PSUM"))
        inT_pool = ctx.enter_context(tc.tile_pool(name="inT", bufs=inT_bufs))
        opsum_pool = ctx.enter_context(tc.tile_pool(name="opsum", bufs=op_bufs, space="PSUM"))
        ep_pool = ctx.enter_context(tc.tile_pool(name="ep", bufs=2))
        noise_pool = ctx.enter_context(tc.tile_pool(name="noise", bufs=2))
        outp_pool = ctx.enter_context(tc.tile_pool(name="outp", bufs=2))

        ident = const_pool.tile([128, 128], dt.float32)
        masks.make_identity(nc, ident[:])
        # wcat[:, k*6 : k*6+6] = [w_gate | w_noise] rows k*128 .. k*128+127
        wcat = const_pool.tile([128, 12], dt.float32)
        for k in range(2):
            nc.sync.dma_start(wcat[:, k * 6 : k * 6 + 3], wg[k * 128 : (k + 1) * 128, :])
            nc.sync.dma_start(
                wcat[:, k * 6 + 3 : k * 6 + 6], wn[k * 128 : (k + 1) * 128, :]
            )

        def build_supergroup(g, noise_all=None, out_sb=None):
            if fuse_io:
                noise_t = noise_all[:, g * ST * E : (g + 1) * ST * E]
            else:
                noise_tile = noise_pool.tile([128, ST * E], dt.float32)
                nc.sync.dma_start(noise_tile[:], noi_r[g])
                noise_t = noise_tile[:]
            psum_out = opsum_pool.tile([128, ST * 6], dt.float32)
            tdt = dt.float32r if tp_f32r else dt.float32
            idn = ident[:].bitcast(dt.float32r) if tp_f32r else ident[:]
            for b in range(blk):
                in_t = in_pool.tile([128, jb * D], tdt)
                eng = getattr(nc, dma_engines[(g * blk + b) % len(dma_engines)])
                src = inp_r[g, b]
                if tp_f32r:
                    src = src.bitcast(dt.float32r)
                eng.dma_start(in_t[:], src)
                for j0 in range(0, jb, tp_batch):
                    if not do_transpose:
                        continue
                    nsub = min(tp_batch, jb - j0)
                    psum_t = tpsum_pool.tile([128, 256 * nsub], tdt)
                    for u in range(nsub):
                        j = j0 + u
                        nc.tensor.transpose(
                            psum_t[:, u * 256 : u * 256 + 128],
                            in_t[:, j * D : j * D + 128],
                            idn,
                        )
                        nc.tensor.transpose(
                            psum_t[:, u * 256 + 128 : u * 256 + 256],
                            in_t[:, j * D + 128 : j * D + 256],
                            idn,
                        )
                    inT = inT_pool.tile([128, 256 * nsub], dt.float32)
                    if copy_split == "any":
                        nc.any.tensor_copy(inT[:], psum_t[:])
                    elif (b * jb + j0) // tp_batch % 2 == 0:
                        nc.vector.tensor_copy(inT[:], psum_t[:])
                    else:
                        nc.scalar.copy(inT[:], psum_t[:])
                    if not do_matmul:
                        continue
                    for u in range(nsub):
                        st = b * jb + j0 + u
                        nc.tensor.matmul(
                            psum_out[:, st * 6 : st * 6 + 6],
                            lhsT=inT[:, u * 256 : u * 256 + 128],
                            rhs=wcat[:, 0:6],
                            start=True,
                            stop=False,
                        )
                        nc.tensor.matmul(
                            psum_out[:, st * 6 : st * 6 + 6],
                            lhsT=inT[:, u * 256 + 128 : u * 256 + 256],
                            rhs=wcat[:, 6:12],
                            start=False,
                            stop=True,
                        )

            if not do_epilogue:
                return
            # epilogue: psum_out [128, 64*6]; per group of 6: [clean0..2, raw0..2]
            p6 = psum_out[:].rearrange("p (s y) -> p s y", y=6)
            clean3 = p6[:, :, 0:3]
            raw3 = p6[:, :, 3:6]
            # softplus via Exp/Ln (both in the natural_log_exp_and_others
            # ACT table). stable: relu(x)+ln(1+exp(-|x|)); naive: ln(1+exp(x))
            sm = nc.vector if small_engine == "vector" else nc.gpsimd
            if softplus_mode == "stable":
                ab = ep_pool.tile([128, ST * E], dt.float32)
                ab3 = ab[:].rearrange("p (s e) -> p s e", e=3)
                nc.scalar.activation(ab3, raw3, Act.Abs)
                ex = ep_pool.tile([128, ST * E], dt.float32)
                nc.scalar.activation(ex[:], ab[:], Act.Exp, scale=-1.0)
                ln1p = ep_pool.tile([128, ST * E], dt.float32)
                nc.scalar.activation(ln1p[:], ex[:], Act.Ln, bias=1.0)
                sp = ep_pool.tile([128, ST * E], dt.float32)
                nc.vector.scalar_tensor_tensor(
                    sp[:].rearrange("p (s e) -> p s e", e=3),
                    raw3,
                    0.0,
                    ln1p[:].rearrange("p (s e) -> p s e", e=3),
                    Alu.max,
                    Alu.add,
                )
            else:
                ex = ep_pool.tile([128, ST * E], dt.float32)
                nc.scalar.activation(
                    ex[:].rearrange("p (s e) -> p s e", e=3), raw3, Act.Exp
                )
                sp = ep_pool.tile([128, ST * E], dt.float32)
                nc.scalar.activation(sp[:], ex[:], Act.Ln, bias=1.0)
            tt = ep_pool.tile([128, ST * E], dt.float32)
            nc.vector.scalar_tensor_tensor(
                tt[:], sp[:], NOISE_EPS, noise_t, Alu.add, Alu.mult
            )
            logits = ep_pool.tile([128, ST * E], dt.float32)
            lg3 = logits[:].rearrange("p (s e) -> p s e", e=3)
            nc.vector.tensor_tensor(
                lg3, clean3, tt[:].rearrange("p (s e) -> p s e", e=3), Alu.add
            )
            l0, l1, l2 = lg3[:, :, 0], lg3[:, :, 1], lg3[:, :, 2]
            c1 = ep_pool.tile([128, ST], dt.float32)
            sm.tensor_tensor(c1[:], l1, l0, Alu.is_gt)
            mx = ep_pool.tile([128, ST], dt.float32)
            sm.tensor_tensor(mx[:], l1, l0, Alu.max)
            c2 = ep_pool.tile([128, ST], dt.float32)
            sm.tensor_tensor(c2[:], l2, mx[:], Alu.is_gt)
            # argmax: idx = max(c1, 2*c2); ties resolve to the earlier expert
            idxf = ep_pool.tile([128, ST], dt.float32)
            sm.scalar_tensor_tensor(
                idxf[:], c2[:], 2.0, c1[:], Alu.mult, Alu.max
            )
            if fuse_io:
                sm.tensor_copy(out_sb[:, g * ST : (g + 1) * ST], idxf[:])
            else:
                idxi = outp_pool.tile([128, ST], dt.int32)
                sm.tensor_copy(idxi[:], idxf[:])
                nc.sync.dma_start(out_r[g], idxi[:])

        def build_iteration():
            noise_all = out_sb = None
            if fuse_io:
                noise_all = noise_pool.tile([128, SG * ST * E], dt.float32)
                nc.sync.dma_start(
                    noise_all[:].rearrange("p (g x) -> p g x", g=SG), noi_f
                )
                out_sb = outp_pool.tile([128, SG * ST], dt.int32)
            for g in range(SG):
                build_supergroup(g, noise_all, out_sb)
            if fuse_io:
                nc.sync.dma_start(
                    out_f, out_sb[:].rearrange("p (g s) -> p g s", g=SG)
                )

        if repeat > 1:
            with tc.For_i(0, repeat, 1):
                build_iteration()
        else:
            build_iteration()

    nc.compile()
    return nc


BEST = dict(
    softplus_mode="stable",
    copy_split="any",
    small_engine="vector",
    jb=8,
    in_bufs=4,
    dma_engines=("sync", "scalar"),
)


def _get_nc():
    if "nc" not in _CACHE:
        _CACHE["nc"] = _build(**BEST)
    return _CACHE["nc"]


def _run(in_maps, trace=False):
    from concourse.bass_utils import run_bass_kernel_spmd

    nc = _get_nc()
    return run_bass_kernel_spmd(nc, in_maps, list(range(NCORES)), trace=trace)


def _make_in_maps(input, w_gate, w_noise, noise):
    input = np.ascontiguousarray(np.asarray(input, dtype=np.float32))
    noise = np.ascontiguousarray(np.asarray(noise, dtype=np.float32))
    w_gate = np.ascontiguousarray(np.asarray(w_gate, dtype=np.float32))
    w_noise = np.ascontiguousarray(np.asarray(w_noise, dtype=np.float32))
    in_maps = []
    for c in range(NCORES):
        sl = slice(c * NPC, (c + 1) * NPC)
        in_maps.append(
            {
                "input": np.ascontiguousarray(input[sl]),
                "noise": np.ascontiguousarray(noise[sl]),
                "w_gate": w_gate,
                "w_noise": w_noise,
            }
        )
    return in_maps


def kernel(input, w_gate, w_noise, noise):
    res = _run(_make_in_maps(input, w_gate, w_noise, noise))
    return np.concatenate([r["out"] for r in res.results], axis=0).astype(np.int32)



# revision 12
# speedup vs baseline: 4.7715x; 4.7715x over previous
"""MoE top-1 routing kernel for Trainium2 (8 NeuronCores, data-parallel).

Computes, for each token t:
    clean   = input[t] @ w_gate                    # [3]
    raw     = input[t] @ w_noise                   # [3]
    logits  = clean + noise[t] * (softplus(raw) + 0.2)
    out[t]  = argmax(logits)                       # int32, first-max tie-break

Sharding: token dim split evenly across 8 cores; [256,3] weights replicated.

Per-core dataflow (32768 tokens):
  - tokens processed in 4 super-groups of 8192; token n = g*8192 + p*64 + s
    (p = SBUF partition, s = sub-tile index) so noise / output DMAs are
    contiguous per partition.
  - input tiles [128 tok, 256 d] are PE-transposed (via identity matmul) to
    [256 d, 128 tok], copied PSUM->SBUF, then matmul'd against the
    concatenated [256, 6] (w_gate | w_noise) weights, accumulating over the
    two 128-row K chunks into PSUM [128 tok, 6] slices.
  - epilogue per super-group: softplus on ACT, noise scale + add + 3-way
    argmax as arithmetic on DVE, cast to int32, DMA out.
"""

import numpy as np

N = 262144
D = 256
E = 3
NCORES = 8
NPC = N // NCORES          # 32768 tokens per core
SG = 4                     # super-groups per core
ST = 64                    # 128-token sub-tiles per super-group
BLK = 8                    # input-DMA blocks per super-group
JB = 8                     # sub-tiles per input-DMA block (1 MiB per DMA)
NOISE_EPS = 0.2

_CACHE = {}


def _build(variant="full", repeat=1, softplus_mode="stable", copy_split="any",
           small_engine="vector", in_bufs=3, tp_bufs=4, inT_bufs=6, op_bufs=2,
           jb=None, dma_engines=("sync",), tp_batch=1, fuse_io=False,
           tp_f32r=False, mm_f32r=False, tp_strips=0, mm_mode="tstat",
           p6_bufs=2, sb6_bufs=3):
    from contextlib import ExitStack

    import concourse.bacc as bacc
    import concourse.mybir as mybir
    import concourse.tile as tile
    from concourse import masks

    dt = mybir.dt
    Alu = mybir.AluOpType
    Act = mybir.ActivationFunctionType
    do_transpose = variant in ("full", "no_epilogue", "no_matmul")
    do_matmul = variant in ("full", "no_epilogue")
    do_epilogue = variant == "full"

    nc = bacc.Bacc(
        "TRN2",
        target_bir_lowering=False,
        debug=False,
        enable_asserts=False,
        num_devices=NCORES,
    )
    inp = nc.dram_tensor("input", [NPC, D], dt.float32, kind="ExternalInput").ap()
    wg = nc.dram_tensor("w_gate", [D, E], dt.float32, kind="ExternalInput").ap()
    wn = nc.dram_tensor("w_noise", [D, E], dt.float32, kind="ExternalInput").ap()
    noi = nc.dram_tensor("noise", [NPC, E], dt.float32, kind="ExternalInput").ap()
    out = nc.dram_tensor("out", [NPC], dt.int32, kind="ExternalOutput").ap()

    jb = JB if jb is None else jb
    blk = ST // jb
    # token n = g*8192 + p*64 + b*jb + j  (p: partition, st = b*jb+j: sub-tile)
    inp_r = inp.rearrange("(g p b j) d -> g b p (j d)", g=SG, p=128, b=blk, j=jb)
    noi_r = noi.rearrange("(g p s) e -> g p (s e)", g=SG, p=128, s=ST)
    out_r = out.rearrange("(g p s) -> g p s", g=SG, p=128, s=ST)
    # fused-I/O layouts: partition p, free (g, s[, e])
    noi_f = noi.rearrange("(g p s) e -> p g (s e)", g=SG, p=128, s=ST)
    out_f = out.rearrange("(g p s) -> p g s", g=SG, p=128, s=ST)

    with tile.TileContext(nc) as tc, ExitStack() as ctx:
        const_pool = ctx.enter_context(tc.tile_pool(name="const", bufs=1))
        in_pool = ctx.enter_context(tc.tile_pool(name="inp", bufs=in_bufs))
        tpsum_pool = ctx.enter_context(tc.tile_pool(name="tpsum", bufs=tp_bufs, space="PSUM"))
        inT_pool = ctx.enter_context(tc.tile_pool(name="inT", bufs=inT_bufs))
        opsum_pool = ctx.enter_context(tc.tile_pool(name="opsum", bufs=op_bufs, space="PSUM"))
        ep_pool = ctx.enter_context(tc.tile_pool(name="ep", bufs=2))
        noise_pool = ctx.enter_context(tc.tile_pool(name="noise", bufs=2))
        outp_pool = ctx.enter_context(tc.tile_pool(name="outp", bufs=2))
        if mm_mode == "wstat":
            p6_pool = ctx.enter_context(
                tc.tile_pool(name="p6", bufs=p6_bufs, space="PSUM")
            )
            sb6_pool = ctx.enter_context(tc.tile_pool(name="sb6", bufs=sb6_bufs))

        ident = const_pool.tile([128, 128], dt.float32)
        masks.make_identity(nc, ident[:])
        # wcat[:, k*6 : k*6+6] = [w_gate | w_noise] rows k*128 .. k*128+127
        wcat_f32 = const_pool.tile([128, 12], dt.float32)
        for k in range(2):
            nc.sync.dma_start(
                wcat_f32[:, k * 6 : k * 6 + 3], wg[k * 128 : (k + 1) * 128, :]
            )
            nc.sync.dma_start(
                wcat_f32[:, k * 6 + 3 : k * 6 + 6], wn[k * 128 : (k + 1) * 128, :]
            )
        if mm_f32r:
            # FP32r operands must be *rounded* by their producer; a DVE copy
            # with f32r out dtype does that.
            wcat = const_pool.tile([128, 12], dt.float32r)
            nc.vector.tensor_copy(wcat[:], wcat_f32[:])
        else:
            wcat = wcat_f32

        def build_supergroup(g, noise_all=None, out_sb=None):
            if fuse_io:
                noise_t = noise_all[:, g * ST * E : (g + 1) * ST * E]
            else:
                noise_tile = noise_pool.tile([128, ST * E], dt.float32)
                nc.sync.dma_start(noise_tile[:], noi_r[g])
                noise_t = noise_tile[:]
            psum_out = opsum_pool.tile([128, ST * 6], dt.float32)
            tdt = dt.float32r if tp_f32r else dt.float32
            idn = ident[:].bitcast(dt.float32r) if tp_f32r else ident[:]
            for b in range(blk):
                in_t = in_pool.tile([128, jb * D], tdt)
                eng = getattr(nc, dma_engines[(g * blk + b) % len(dma_engines)])
                src = inp_r[g, b]
                if tp_f32r:
                    src = src.bitcast(dt.float32r)
                eng.dma_start(in_t[:], src)
                for j0 in range(0, jb, tp_batch):
                    if not do_transpose:
                        continue
                    nsub = min(tp_batch, jb - j0)
                    psum_t = tpsum_pool.tile([128, 256 * nsub], tdt)
                    for u in range(nsub):
                        j = j0 + u
                        for k in range(2):
                            src = in_t[:, j * D + 128 * k : j * D + 128 * (k + 1)]
                            if mm_mode == "wstat":
                                # d-chunk-outer layout: chunk k's tokens are
                                # contiguous so the wstat matmul can stream
                                # them as one 128*nsub-row moving operand.
                                dst = psum_t[
                                    :, k * 128 * nsub + u * 128 : k * 128 * nsub + u * 128 + 128
                                ]
                            else:
                                dst = psum_t[:, u * 256 + 128 * k : u * 256 + 128 * (k + 1)]
                            if tp_strips:
                                sw = 128 // tp_strips
                                for s in range(tp_strips):
                                    nc.tensor.transpose(
                                        dst[s * sw : (s + 1) * sw, :],
                                        src[:, s * sw : (s + 1) * sw],
                                        idn,
                                        tile_position=(0, s * sw),
                                    )
                            else:
                                nc.tensor.transpose(dst, src, idn)
                    inT = inT_pool.tile(
                        [128, 256 * nsub], dt.float32r if mm_f32r else dt.float32
                    )
                    cp_src = psum_t[:]
                    if tp_f32r and mm_f32r:
                        # transpose ran in f32r (bit-exact move); read the psum
                        # back as f32 so this copy is the f32->f32r *rounding*
                        # producer the BIR verifier requires for the matmul.
                        cp_src = cp_src.bitcast(dt.float32)
                    if copy_split == "any":
                        nc.any.tensor_copy(inT[:], cp_src)
                    elif (b * jb + j0) // tp_batch % 2 == 0:
                        nc.vector.tensor_copy(inT[:], cp_src)
                    else:
                        nc.scalar.copy(inT[:], cp_src)
                    if not do_matmul:
                        continue
                    if mm_mode == "wstat":
                        ntk = 128 * nsub
                        p6 = p6_pool.tile([6, ntk], dt.float32)
                        nc.tensor.matmul(
                            p6[:],
                            lhsT=wcat[:, 0:6],
                            rhs=inT[:, 0:ntk],
                            start=True,
                            stop=False,
                        )
                        nc.tensor.matmul(
                            p6[:],
                            lhsT=wcat[:, 6:12],
                            rhs=inT[:, ntk : 2 * ntk],
                            start=False,
                            stop=True,
                        )
                        sb6 = sb6_pool.tile([6, ntk], dt.float32)
                        nc.any.tensor_copy(sb6[:], p6[:])
                        for u in range(nsub):
                            st = b * jb + j0 + u
                            nc.tensor.transpose(
                                psum_out[:, st * 6 : st * 6 + 6],
                                sb6[:, u * 128 : (u + 1) * 128],
                                ident[0:6, 0:6],
                            )
                        continue
                    for u in range(nsub):
                        st = b * jb + j0 + u
                        lhs0 = inT[:, u * 256 : u * 256 + 128]
                        lhs1 = inT[:, u * 256 + 128 : u * 256 + 256]
                        rhs0 = wcat[:, 0:6]
                        rhs1 = wcat[:, 6:12]
                        nc.tensor.matmul(
                            psum_out[:, st * 6 : st * 6 + 6],
                            lhsT=lhs0,
                            rhs=rhs0,
                            start=True,
                            stop=False,
                        )
                        nc.tensor.matmul(
                            psum_out[:, st * 6 : st * 6 + 6],
                            lhsT=lhs1,
                            rhs=rhs1,
                            start=False,
                            stop=True,
                        )

            if not do_epilogue:
                return
            # epilogue: psum_out [128, 64*6]; per group of 6: [clean0..2, raw0..2]
            p6 = psum_out[:].rearrange("p (s y) -> p s y", y=6)
            clean3 = p6[:, :, 0:3]
            raw3 = p6[:, :, 3:6]
            # softplus via Exp/Ln (both in the natural_log_exp_and_others
            # ACT table). stable: relu(x)+ln(1+exp(-|x|)); naive: ln(1+exp(x))
            sm = nc.vector if small_engine == "vector" else nc.gpsimd
            if softplus_mode == "stable":
                ab = ep_pool.tile([128, ST * E], dt.float32)
                ab3 = ab[:].rearrange("p (s e) -> p s e", e=3)
                nc.scalar.activation(ab3, raw3, Act.Abs)
                ex = ep_pool.tile([128, ST * E], dt.float32)
                nc.scalar.activation(ex[:], ab[:], Act.Exp, scale=-1.0)
                ln1p = ep_pool.tile([128, ST * E], dt.float32)
                nc.scalar.activation(ln1p[:], ex[:], Act.Ln, bias=1.0)
                sp = ep_pool.tile([128, ST * E], dt.float32)
                nc.vector.scalar_tensor_tensor(
                    sp[:].rearrange("p (s e) -> p s e", e=3),
                    raw3,
                    0.0,
                    ln1p[:].rearrange("p (s e) -> p s e", e=3),
                    Alu.max,
                    Alu.add,
                )
            else:
                ex = ep_pool.tile([128, ST * E], dt.float32)
                nc.scalar.activation(
                    ex[:].rearrange("p (s e) -> p s e", e=3), raw3, Act.Exp
                )
                sp = ep_pool.tile([128, ST * E], dt.float32)
                nc.scalar.activation(sp[:], ex[:], Act.Ln, bias=1.0)
            tt = ep_pool.tile([128, ST * E], dt.float32)
            nc.vector.scalar_tensor_tensor(
                tt[:], sp[:], NOISE_EPS, noise_t, Alu.add, Alu.mult
            )
            logits = ep_pool.tile([128, ST * E], dt.float32)
            lg3 = logits[:].rearrange("p (s e) -> p s e", e=3)
            nc.vector.tensor_tensor(
                lg3, clean3, tt[:].rearrange("p (s e) -> p s e", e=3), Alu.add
            )
            l0, l1, l2 = lg3[:, :, 0], lg3[:, :, 1], lg3[:, :, 2]
            c1 = ep_pool.tile([128, ST], dt.float32)
            sm.tensor_tensor(c1[:], l1, l0, Alu.is_gt)
            mx = ep_pool.tile([128, ST], dt.float32)
            sm.tensor_tensor(mx[:], l1, l0, Alu.max)
            c2 = ep_pool.tile([128, ST], dt.float32)
            sm.tensor_tensor(c2[:], l2, mx[:], Alu.is_gt)
            # argmax: idx = max(c1, 2*c2); ties resolve to the earlier expert
            idxf = ep_pool.tile([128, ST], dt.float32)
            sm.scalar_tensor_tensor(
                idxf[:], c2[:], 2.0, c1[:], Alu.mult, Alu.max
            )
            if fuse_io:
                sm.tensor_copy(out_sb[:, g * ST : (g + 1) * ST], idxf[:])
            else:
                idxi = outp_pool.tile([128, ST], dt.int32)
                sm.tensor_copy(idxi[:], idxf[:])
                nc.sync.dma_start(out_r[g], idxi[:])

        def build_iteration():
            noise_all = out_sb = None
            if fuse_io:
                noise_all = noise_pool.tile([128, SG * ST * E], dt.float32)
                nc.sync.dma_start(
                    noise_all[:].rearrange("p (g x) -> p g x", g=SG), noi_f
                )
                out_sb = outp_pool.tile([128, SG * ST], dt.int32)
            for g in range(SG):
                build_supergroup(g, noise_all, out_sb)
            if fuse_io:
                nc.sync.dma_start(
                    out_f, out_sb[:].rearrange("p (g s) -> p g s", g=SG)
                )

        if repeat > 1:
            with tc.For_i(0, repeat, 1):
                build_iteration()
        else:
            build_iteration()

    nc.compile()
    return nc


BEST = dict(
    softplus_mode="stable",
    copy_split="any",
    small_engine="vector",
    jb=8,
    in_bufs=4,
    dma_engines=("sync", "scalar"),
    mm_f32r=True,
    tp_batch=2,
)


def _get_nc():
    if "nc" not in _CACHE:
        _CACHE["nc"] = _build(**BEST)
    return _CACHE["nc"]


def _run(in_maps, trace=False):
    from concourse.bass_utils import run_bass_kernel_spmd

    nc = _get_nc()
    return run_bass_kernel_spmd(nc, in_maps, list(range(NCORES)), trace=trace)


def _make_in_maps(input, w_gate, w_noise, noise):
    input = np.ascontiguousarray(np.asarray(input, dtype=np.float32))
    noise = np.ascontiguousarray(np.asarray(noise, dtype=np.float32))
    w_gate = np.ascontiguousarray(np.asarray(w_gate, dtype=np.float32))
    w_noise = np.ascontiguousarray(np.asarray(w_noise, dtype=np.float32))
    in_maps = []
    for c in range(NCORES):
        sl = slice(c * NPC, (c + 1) * NPC)
        in_maps.append(
            {
                "input": np.ascontiguousarray(input[sl]),
                "noise": np.ascontiguousarray(noise[sl]),
                "w_gate": w_gate,
                "w_noise": w_noise,
            }
        )
    return in_maps


def kernel(input, w_gate, w_noise, noise):
    res = _run(_make_in_maps(input, w_gate, w_noise, noise))
    return np.concatenate([r["out"] for r in res.results], axis=0).astype(np.int32)

